# revision 1
# baseline (speedup 1.0000x reference)
"""Trainium2 Bass kernel for masked-dropout attention-score matmul.

Computes, for q/k/v [B,H,S,D] and an int32 0/1 keep-mask [B,H,S,S]:

    out = ((q @ k^T) * sqrt(D) * 2 * mask) @ v        (2 = 1/(1-p_drop))

Strategy (8 NeuronCores, SPMD, no collectives):
  - Shard the 32 (b,h) pairs 4-per-core.
  - Per pair, compute S^T = K @ Q^T on the PE (so the second matmul can
    consume it as its moving operand without any on-chip transpose),
    apply the mask fused into the PSUM->SBUF eviction on the DVE, and
    accumulate O^T = V^T @ S'^T on the PE.
  - The scale (2*sqrt(D)) is folded into V on the host; mask values are
    shipped as fp8(0/1) bytes; Q^T/K^T/V are host-rearranged so all
    device DMAs are fully contiguous.
"""

import os
import sys

sys.path.insert(0, "/opt/trn_rl_repo")

import numpy as np

import concourse.bacc as bacc
import concourse.bass as bass
import concourse.mybir as mybir
import concourse.tile as tile
from concourse.bass_utils import run_bass_kernel_spmd

B, H, SQ, SK, D = 2, 16, 2048, 2048, 128
P_DROP = 0.5
SCALE = float(D) ** 0.5 / (1.0 - P_DROP)  # folded into V on the host
N_CORES = 8
PAIRS = B * H
PAIRS_PER_CORE = PAIRS // N_CORES

F32 = mybir.dt.float32
F32R = mybir.dt.float32r
FP8 = mybir.dt.float8e4
U8 = mybir.dt.uint8
BF16 = mybir.dt.bfloat16

FP8_ONE = 0x38  # float8_e4m3 encoding of 1.0

# module-level handle for test.py to inspect timing after a traced run
LAST_RESULTS = None


def emit_body(nc, tc, ot, qt, kt, v, mt, n_pairs, sq, sk, d=D, qn=512, repeat=1,
              loop_n=1, mmdt=F32R):
    """Emit the per-core program.

    APs (all on this core's DRAM):
      qt [n_pairs, d,  sq]  f32  : Q^T per pair
      kt [n_pairs, d,  sk]  f32  : K^T per pair
      v  [n_pairs, d?, ...]      : V rearranged to [128, (sk//128)*d], f32,
                                   v[p][r][c*d+j] = V[c*128+r, j] * SCALE
      mt [n_pairs, sk, sq]  u8   : mask^T as fp8 bytes (0x00 / 0x38)
      ot [n_pairs, d,  sq]  f32  : O^T output
    """
    nkc = sk // 128
    nqc = sq // qn

    import contextlib

    with contextlib.ExitStack() as ctx:
        qt_pool = ctx.enter_context(tc.tile_pool(name="qt", bufs=2))
        kt_pool = ctx.enter_context(tc.tile_pool(name="kt", bufs=2))
        v_pool = ctx.enter_context(tc.tile_pool(name="v", bufs=2))
        m_pool = ctx.enter_context(tc.tile_pool(name="m", bufs=4))
        sp_pool = ctx.enter_context(tc.tile_pool(name="sp", bufs=6))
        o_pool = ctx.enter_context(tc.tile_pool(name="o", bufs=2))
        st_pool = ctx.enter_context(tc.tile_pool(name="st", bufs=4, space="PSUM"))
        ot_pool = ctx.enter_context(tc.tile_pool(name="otp", bufs=1, space="PSUM"))

        loop_cm = tc.For_i(0, loop_n, 1) if loop_n > 1 else contextlib.nullcontext()
        with loop_cm:
          for p in [pp for _ in range(repeat) for pp in range(n_pairs)]:
            qt_t = qt_pool.tile([128, sq], mmdt)
            nc.sync.dma_start(out=qt_t[:d], in_=qt[p])
            kt_t = kt_pool.tile([128, sk], mmdt)
            nc.sync.dma_start(out=kt_t[:d], in_=kt[p])
            v_t = v_pool.tile([128, nkc * d], mmdt)
            nc.sync.dma_start(out=v_t[:], in_=v[p])

            ot_ps = ot_pool.tile([128, sq], F32)

            for kc in range(nkc):
                m_t = m_pool.tile([128, sq], U8)
                nc.sync.dma_start(out=m_t[:], in_=mt[p, kc * 128 : (kc + 1) * 128, :])

                for qc in range(nqc):
                    st = st_pool.tile([128, qn], F32)
                    nc.tensor.matmul(
                        st[:],
                        kt_t[:d, kc * 128 : (kc + 1) * 128],
                        qt_t[:d, qc * qn : (qc + 1) * qn],
                        start=True,
                        stop=True,
                    )
                    sp = sp_pool.tile([128, qn], mmdt)
                    nc.vector.tensor_mul(
                        sp[:],
                        st[:],
                        m_t[:, qc * qn : (qc + 1) * qn].bitcast(FP8),
                    )
                    nc.tensor.matmul(
                        ot_ps[:d, qc * qn : (qc + 1) * qn],
                        v_t[:, kc * d : (kc + 1) * d],
                        sp[:],
                        start=(kc == 0),
                        stop=(kc == nkc - 1),
                    )

            o_t = o_pool.tile([128, sq], F32)
            nc.scalar.copy(o_t[:d], ot_ps[:d])
            nc.sync.dma_start(out=ot[p], in_=o_t[:d])


def emit_body_v2(
    nc, tc, ot, qt, kt, v, mt, n_pairs, sq, sk, d=D, qn=512, gn=1024, fused_mod=(1, 4),
    repeat=1, loop_n=1,
):
    """Balanced-engine variant.

    Masking is split across three engines per [128, gn] score group:
      - fused path (idx % fused_mod[1] < fused_mod[0]): DVE multiplies
        PSUM f32 scores by the fp8 mask directly -> bf16 SBUF.
      - split path: ACT evicts PSUM f32 -> bf16 SBUF, GpSimd converts the
        fp8 mask -> bf16, DVE multiplies bf16 x bf16 in its 2x mode.
    Second matmul runs with bf16 moving operand at N=gn; V ships as bf16.
    """
    nkc = sk // 128
    ngc = sq // gn

    import contextlib

    with contextlib.ExitStack() as ctx:
        qt_pool = ctx.enter_context(tc.tile_pool(name="qt", bufs=2))
        kt_pool = ctx.enter_context(tc.tile_pool(name="kt", bufs=2))
        v_pool = ctx.enter_context(tc.tile_pool(name="v", bufs=2))
        m_pool = ctx.enter_context(tc.tile_pool(name="m", bufs=4))
        sp_pool = ctx.enter_context(tc.tile_pool(name="sp", bufs=6))
        se_pool = ctx.enter_context(tc.tile_pool(name="se", bufs=4))
        mb_pool = ctx.enter_context(tc.tile_pool(name="mb", bufs=4))
        o_pool = ctx.enter_context(tc.tile_pool(name="o", bufs=2))
        st_pool = ctx.enter_context(tc.tile_pool(name="st", bufs=2, space="PSUM"))
        ot_pool = ctx.enter_context(tc.tile_pool(name="otp", bufs=1, space="PSUM"))

        unit = 0
        loop_cm = tc.For_i(0, loop_n, 1) if loop_n > 1 else contextlib.nullcontext()
        with loop_cm:
          for p in [pp for _ in range(repeat) for pp in range(n_pairs)]:
            qt_t = qt_pool.tile([128, sq], F32R)
            nc.sync.dma_start(out=qt_t[:d], in_=qt[p])
            kt_t = kt_pool.tile([128, sk], F32R)
            nc.sync.dma_start(out=kt_t[:d], in_=kt[p])
            v_t = v_pool.tile([128, nkc * d], BF16)
            nc.sync.dma_start(out=v_t[:], in_=v[p])

            ot_ps = ot_pool.tile([128, sq], F32)

            for kc in range(nkc):
                m_t = m_pool.tile([128, sq], U8)
                nc.sync.dma_start(out=m_t[:], in_=mt[p, kc * 128 : (kc + 1) * 128, :])

                for g in range(ngc):
                    st = st_pool.tile([128, gn], F32)
                    for j in range(gn // qn):
                        c0 = g * gn + j * qn
                        nc.tensor.matmul(
                            st[:, j * qn : (j + 1) * qn],
                            kt_t[:d, kc * 128 : (kc + 1) * 128],
                            qt_t[:d, c0 : c0 + qn],
                            start=True,
                            stop=True,
                        )
                    m_sl = m_t[:, g * gn : (g + 1) * gn].bitcast(FP8)
                    sp = sp_pool.tile([128, gn], BF16)
                    if unit % fused_mod[1] < fused_mod[0]:
                        nc.vector.tensor_mul(sp[:], st[:], m_sl)
                    else:
                        se = se_pool.tile([128, gn], BF16)
                        nc.scalar.copy(se[:], st[:])
                        mb = mb_pool.tile([128, gn], BF16)
                        nc.gpsimd.tensor_copy(mb[:], m_sl)
                        nc.vector.tensor_mul(sp[:], se[:], mb[:])
                    unit += 1
                    for j in range(gn // qn):
                        c0 = g * gn + j * qn
                        nc.tensor.matmul(
                            ot_ps[:d, c0 : c0 + qn],
                            v_t[:, kc * d : (kc + 1) * d],
                            sp[:, j * qn : (j + 1) * qn],
                            start=(kc == 0),
                            stop=(kc == nkc - 1),
                        )

            o_t = o_pool.tile([128, sq], F32)
            nc.scalar.copy(o_t[:d], ot_ps[:d])
            nc.sync.dma_start(out=ot[p], in_=o_t[:d])


def build_nc(n_pairs=PAIRS_PER_CORE, sq=SQ, sk=SK, d=D, qn=512, variant="v1", repeat=1,
             loop_n=1):
    nc = bacc.Bacc("TRN2", target_bir_lowering=False, debug=False)
    mmdt = F32R if variant == "v1" else BF16
    vdt = mmdt
    qt = nc.declare_dram_parameter("qt", [n_pairs, d, sq], mmdt, isOutput=False)
    kt = nc.declare_dram_parameter("kt", [n_pairs, d, sk], mmdt, isOutput=False)
    v = nc.declare_dram_parameter("v", [n_pairs, 128, (sk // 128) * d], vdt, isOutput=False)
    mt = nc.declare_dram_parameter("mt", [n_pairs, sk, sq], U8, isOutput=False)
    ot = nc.declare_dram_parameter("ot", [n_pairs, d, sq], F32, isOutput=True)
    with tile.TileContext(nc) as tc:
        if variant in ("v1", "v3"):
            emit_body(nc, tc, ot, qt, kt, v, mt, n_pairs, sq, sk, d, qn, repeat=repeat,
                      loop_n=loop_n, mmdt=mmdt)
        else:
            emit_body_v2(nc, tc, ot, qt, kt, v, mt, n_pairs, sq, sk, d, qn,
                         repeat=repeat, loop_n=loop_n)
    nc.compile()
    return nc


def _prep_inputs(query, key, value, dropout_mask, variant="v1"):
    """Host-side marshaling into per-core input maps."""
    import ml_dtypes

    q = np.asarray(query, dtype=np.float32).reshape(PAIRS, SQ, D)
    k = np.asarray(key, dtype=np.float32).reshape(PAIRS, SK, D)
    vv = np.asarray(value, dtype=np.float32).reshape(PAIRS, SK, D)
    m = np.asarray(dropout_mask).reshape(PAIRS, SQ, SK)

    qt = np.ascontiguousarray(q.transpose(0, 2, 1))  # [PAIRS, D, SQ]
    kt = np.ascontiguousarray(k.transpose(0, 2, 1))  # [PAIRS, D, SK]
    # V * SCALE rearranged: vr[p][r][c*D+j] = V[c*128+r, j] * SCALE
    vr = (vv * np.float32(SCALE)).reshape(PAIRS, SK // 128, 128, D)
    vr = np.ascontiguousarray(vr.transpose(0, 2, 1, 3)).reshape(PAIRS, 128, (SK // 128) * D)
    if variant != "v1":
        vr = vr.astype(ml_dtypes.bfloat16)
        qt = qt.astype(ml_dtypes.bfloat16)
        kt = kt.astype(ml_dtypes.bfloat16)
    # mask^T as fp8 bytes
    mb = (m != 0).astype(np.uint8) * np.uint8(FP8_ONE)  # [PAIRS, SQ, SK] u8
    mbt = np.ascontiguousarray(mb.transpose(0, 2, 1))  # [PAIRS, SK, SQ]

    in_maps = []
    for c in range(N_CORES):
        s = slice(c * PAIRS_PER_CORE, (c + 1) * PAIRS_PER_CORE)
        in_maps.append(
            {
                "qt": qt[s],
                "kt": kt[s],
                "v": vr[s],
                "mt": mbt[s],
            }
        )
    return in_maps


def kernel(query, key, value, dropout_mask):
    global LAST_RESULTS
    variant = os.environ.get("KERNEL_VARIANT", "v1")
    in_maps = _prep_inputs(query, key, value, dropout_mask, variant)
    nc = build_nc(variant=variant)
    res = run_bass_kernel_spmd(nc, in_maps, list(range(N_CORES)), trace=False)
    LAST_RESULTS = res
    outs = np.concatenate([r["ot"] for r in res.results], axis=0)  # [PAIRS, D, SQ]
    out = outs.transpose(0, 2, 1).reshape(B, H, SQ, D)
    return np.ascontiguousarray(out.astype(np.float32, copy=False))



# revision 8
# speedup vs baseline: 6.0240x; 6.0240x over previous
"""Trainium2 Bass kernel for masked-dropout attention-score matmul.

Computes, for q/k/v [B,H,S,D] and an int32 0/1 keep-mask [B,H,S,S]:

    out = ((q @ k^T) * sqrt(D) * 2 * mask) @ v        (2 = 1/(1-p_drop))

Strategy (8 NeuronCores, SPMD, no collectives):
  - Shard the 32 (b,h) pairs 4-per-core.
  - Per pair, compute S^T = K @ Q^T on the PE (so the second matmul can
    consume it as its moving operand without any on-chip transpose),
    apply the mask fused into the PSUM->SBUF eviction on the DVE, and
    accumulate O^T = V^T @ S'^T on the PE.
  - The scale (2*sqrt(D)) is folded into V on the host; mask values are
    shipped as fp8(0/1) bytes; Q^T/K^T/V are host-rearranged so all
    device DMAs are fully contiguous.
"""

import os
import sys

sys.path.insert(0, "/opt/trn_rl_repo")

import numpy as np

import concourse.bacc as bacc
import concourse.bass as bass
import concourse.mybir as mybir
import concourse.tile as tile
from concourse.bass_utils import run_bass_kernel_spmd

B, H, SQ, SK, D = 2, 16, 2048, 2048, 128
P_DROP = 0.5
SCALE = float(D) ** 0.5 / (1.0 - P_DROP)  # folded into V on the host
N_CORES = 8
PAIRS = B * H
PAIRS_PER_CORE = PAIRS // N_CORES

F32 = mybir.dt.float32
F32R = mybir.dt.float32r
FP8 = mybir.dt.float8e4
U8 = mybir.dt.uint8
BF16 = mybir.dt.bfloat16

FP8_ONE = 0x38  # float8_e4m3 encoding of 1.0

# module-level handle for test.py to inspect timing after a traced run
LAST_RESULTS = None


def emit_body(nc, tc, ot, qt, kt, v, mt, n_pairs, sq, sk, d=D, qn=512, repeat=1,
              loop_n=1, mmdt=F32R):
    """Emit the per-core program.

    APs (all on this core's DRAM):
      qt [n_pairs, d,  sq]  f32  : Q^T per pair
      kt [n_pairs, d,  sk]  f32  : K^T per pair
      v  [n_pairs, d?, ...]      : V rearranged to [128, (sk//128)*d], f32,
                                   v[p][r][c*d+j] = V[c*128+r, j] * SCALE
      mt [n_pairs, sk, sq]  u8   : mask^T as fp8 bytes (0x00 / 0x38)
      ot [n_pairs, d,  sq]  f32  : O^T output
    """
    nkc = sk // 128
    nqc = sq // qn

    import contextlib

    with contextlib.ExitStack() as ctx:
        qt_pool = ctx.enter_context(tc.tile_pool(name="qt", bufs=2))
        kt_pool = ctx.enter_context(tc.tile_pool(name="kt", bufs=2))
        v_pool = ctx.enter_context(tc.tile_pool(name="v", bufs=2))
        m_pool = ctx.enter_context(tc.tile_pool(name="m", bufs=4))
        sp_pool = ctx.enter_context(tc.tile_pool(name="sp", bufs=6))
        o_pool = ctx.enter_context(tc.tile_pool(name="o", bufs=2))
        st_pool = ctx.enter_context(tc.tile_pool(name="st", bufs=4, space="PSUM"))
        ot_pool = ctx.enter_context(tc.tile_pool(name="otp", bufs=1, space="PSUM"))

        loop_cm = tc.For_i(0, loop_n, 1) if loop_n > 1 else contextlib.nullcontext()
        with loop_cm:
          for p in [pp for _ in range(repeat) for pp in range(n_pairs)]:
            qt_t = qt_pool.tile([128, sq], mmdt)
            nc.sync.dma_start(out=qt_t[:d], in_=qt[p])
            kt_t = kt_pool.tile([128, sk], mmdt)
            nc.sync.dma_start(out=kt_t[:d], in_=kt[p])
            v_t = v_pool.tile([128, nkc * d], mmdt)
            nc.sync.dma_start(out=v_t[:], in_=v[p])

            ot_ps = ot_pool.tile([128, sq], F32)

            for kc in range(nkc):
                m_t = m_pool.tile([128, sq], U8)
                nc.sync.dma_start(out=m_t[:], in_=mt[p, kc * 128 : (kc + 1) * 128, :])

                for qc in range(nqc):
                    st = st_pool.tile([128, qn], F32)
                    nc.tensor.matmul(
                        st[:],
                        kt_t[:d, kc * 128 : (kc + 1) * 128],
                        qt_t[:d, qc * qn : (qc + 1) * qn],
                        start=True,
                        stop=True,
                    )
                    sp = sp_pool.tile([128, qn], mmdt)
                    nc.vector.tensor_mul(
                        sp[:],
                        st[:],
                        m_t[:, qc * qn : (qc + 1) * qn].bitcast(FP8),
                    )
                    nc.tensor.matmul(
                        ot_ps[:d, qc * qn : (qc + 1) * qn],
                        v_t[:, kc * d : (kc + 1) * d],
                        sp[:],
                        start=(kc == 0),
                        stop=(kc == nkc - 1),
                    )

            o_t = o_pool.tile([128, sq], F32)
            nc.scalar.copy(o_t[:d], ot_ps[:d])
            nc.sync.dma_start(out=ot[p], in_=o_t[:d])


def emit_body_v2(
    nc, tc, ot, qt, kt, v, mt, n_pairs, sq, sk, d=D, qn=512, gn=1024, fused_mod=(1, 4),
    repeat=1, loop_n=1,
):
    """Balanced-engine variant.

    Masking is split across three engines per [128, gn] score group:
      - fused path (idx % fused_mod[1] < fused_mod[0]): DVE multiplies
        PSUM f32 scores by the fp8 mask directly -> bf16 SBUF.
      - split path: ACT evicts PSUM f32 -> bf16 SBUF, GpSimd converts the
        fp8 mask -> bf16, DVE multiplies bf16 x bf16 in its 2x mode.
    Second matmul runs with bf16 moving operand at N=gn; V ships as bf16.
    """
    nkc = sk // 128
    ngc = sq // gn

    import contextlib

    with contextlib.ExitStack() as ctx:
        qt_pool = ctx.enter_context(tc.tile_pool(name="qt", bufs=2))
        kt_pool = ctx.enter_context(tc.tile_pool(name="kt", bufs=2))
        v_pool = ctx.enter_context(tc.tile_pool(name="v", bufs=2))
        m_pool = ctx.enter_context(tc.tile_pool(name="m", bufs=4))
        sp_pool = ctx.enter_context(tc.tile_pool(name="sp", bufs=6))
        se_pool = ctx.enter_context(tc.tile_pool(name="se", bufs=4))
        mb_pool = ctx.enter_context(tc.tile_pool(name="mb", bufs=4))
        o_pool = ctx.enter_context(tc.tile_pool(name="o", bufs=2))
        st_pool = ctx.enter_context(tc.tile_pool(name="st", bufs=2, space="PSUM"))
        ot_pool = ctx.enter_context(tc.tile_pool(name="otp", bufs=1, space="PSUM"))

        unit = 0
        loop_cm = tc.For_i(0, loop_n, 1) if loop_n > 1 else contextlib.nullcontext()
        with loop_cm:
          for p in [pp for _ in range(repeat) for pp in range(n_pairs)]:
            qt_t = qt_pool.tile([128, sq], F32R)
            nc.sync.dma_start(out=qt_t[:d], in_=qt[p])
            kt_t = kt_pool.tile([128, sk], F32R)
            nc.sync.dma_start(out=kt_t[:d], in_=kt[p])
            v_t = v_pool.tile([128, nkc * d], BF16)
            nc.sync.dma_start(out=v_t[:], in_=v[p])

            ot_ps = ot_pool.tile([128, sq], F32)

            for kc in range(nkc):
                m_t = m_pool.tile([128, sq], U8)
                nc.sync.dma_start(out=m_t[:], in_=mt[p, kc * 128 : (kc + 1) * 128, :])

                for g in range(ngc):
                    st = st_pool.tile([128, gn], F32)
                    for j in range(gn // qn):
                        c0 = g * gn + j * qn
                        nc.tensor.matmul(
                            st[:, j * qn : (j + 1) * qn],
                            kt_t[:d, kc * 128 : (kc + 1) * 128],
                            qt_t[:d, c0 : c0 + qn],
                            start=True,
                            stop=True,
                        )
                    m_sl = m_t[:, g * gn : (g + 1) * gn].bitcast(FP8)
                    sp = sp_pool.tile([128, gn], BF16)
                    if unit % fused_mod[1] < fused_mod[0]:
                        nc.vector.tensor_mul(sp[:], st[:], m_sl)
                    else:
                        se = se_pool.tile([128, gn], BF16)
                        nc.scalar.copy(se[:], st[:])
                        mb = mb_pool.tile([128, gn], BF16)
                        nc.gpsimd.tensor_copy(mb[:], m_sl)
                        nc.vector.tensor_mul(sp[:], se[:], mb[:])
                    unit += 1
                    for j in range(gn // qn):
                        c0 = g * gn + j * qn
                        nc.tensor.matmul(
                            ot_ps[:d, c0 : c0 + qn],
                            v_t[:, kc * d : (kc + 1) * d],
                            sp[:, j * qn : (j + 1) * qn],
                            start=(kc == 0),
                            stop=(kc == nkc - 1),
                        )

            o_t = o_pool.tile([128, sq], F32)
            nc.scalar.copy(o_t[:d], ot_ps[:d])
            nc.sync.dma_start(out=ot[p], in_=o_t[:d])


def emit_body_v4(nc, tc, ot, qt, kt, v, mt, n_pairs, sq, sk, d=D, qn=512,
                 loop_n=1, st_bufs=6, m_bufs=2):
    """qc-outer / kc-inner: PSUM out chunk is one [128, qn] bank (bufs=2),
    evictions overlap compute; all matmul I/O bf16; mask fp8 fused on DVE.

    APs:
      qt [n_pairs, d, sq]   bf16 : Q^T
      kt [n_pairs, d, sk]   bf16 : K^T
      v  [n_pairs, 128, (sk//128)*d] bf16 : V * SCALE, kc-chunked
      mt [n_pairs, sq//qn, 128, (sk//128)*qn] u8 : mask fp8 bytes, laid out
         mt[p, qc, part, kc*qn + j] = mask[k = kc*128+part, q = qc*qn+j]
      ot [n_pairs, sq//qn, d, qn] bf16 : O^T chunks
    """
    nkc = sk // 128
    nqc = sq // qn

    import contextlib

    with contextlib.ExitStack() as ctx:
        qt_pool = ctx.enter_context(tc.tile_pool(name="qt", bufs=2))
        kt_pool = ctx.enter_context(tc.tile_pool(name="kt", bufs=2))
        v_pool = ctx.enter_context(tc.tile_pool(name="v", bufs=2))
        m_pool = ctx.enter_context(tc.tile_pool(name="m", bufs=m_bufs))
        sp_pool = ctx.enter_context(tc.tile_pool(name="sp", bufs=6))
        o_pool = ctx.enter_context(tc.tile_pool(name="o", bufs=4))
        st_pool = ctx.enter_context(tc.tile_pool(name="st", bufs=st_bufs, space="PSUM"))
        ot_pool = ctx.enter_context(tc.tile_pool(name="otp", bufs=2, space="PSUM"))

        loop_cm = tc.For_i(0, loop_n, 1) if loop_n > 1 else contextlib.nullcontext()
        with loop_cm:
          for p in range(n_pairs):
            qt_t = qt_pool.tile([128, sq], BF16)
            nc.sync.dma_start(out=qt_t[:d], in_=qt[p])
            kt_t = kt_pool.tile([128, sk], BF16)
            nc.sync.dma_start(out=kt_t[:d], in_=kt[p])
            v_t = v_pool.tile([128, nkc * d], BF16)
            nc.sync.dma_start(out=v_t[:], in_=v[p])

            for qc in range(nqc):
                m_t = m_pool.tile([128, nkc * qn], U8)
                nc.sync.dma_start(out=m_t[:], in_=mt[p, qc])
                ot_ps = ot_pool.tile([128, qn], F32)

                for kc in range(nkc):
                    st = st_pool.tile([128, qn], F32)
                    nc.tensor.matmul(
                        st[:],
                        kt_t[:d, kc * 128 : (kc + 1) * 128],
                        qt_t[:d, qc * qn : (qc + 1) * qn],
                        start=True,
                        stop=True,
                    )
                    sp = sp_pool.tile([128, qn], BF16)
                    nc.vector.tensor_mul(
                        sp[:],
                        st[:],
                        m_t[:, kc * qn : (kc + 1) * qn].bitcast(FP8),
                    )
                    nc.tensor.matmul(
                        ot_ps[:d],
                        v_t[:, kc * d : (kc + 1) * d],
                        sp[:],
                        start=(kc == 0),
                        stop=(kc == nkc - 1),
                    )

                o_t = o_pool.tile([128, qn], BF16)
                nc.scalar.copy(o_t[:d], ot_ps[:d])
                nc.sync.dma_start(out=ot[p, qc], in_=o_t[:d])


def emit_body_v5(nc, tc, ot, qt, kt, v, mt, n_pairs, sq, sk, d=D, qn=512,
                 loop_n=1, a_every=3, st_bufs=5):
    """3-engine masked eviction, qc-outer/kc-inner.

    Unit classes (u = global unit counter):
      D (default): DVE fused mul-evict  sp_f32r = st_PSUM * mask_fp8
      A (u % a_every == a_every-1): ACT copy st_PSUM -> se_f32, then
        GpSimd mul sp_f32r = se * mask_fp8  (keeps DVE free)
    MM1 bf16 (qt,kt); MM2 f32r moving (sp) x f32r stationary (v).
    """
    nkc = sk // 128
    nqc = sq // qn

    import contextlib

    with contextlib.ExitStack() as ctx:
        qt_pool = ctx.enter_context(tc.tile_pool(name="qt", bufs=2))
        kt_pool = ctx.enter_context(tc.tile_pool(name="kt", bufs=2))
        v_pool = ctx.enter_context(tc.tile_pool(name="v", bufs=2))
        m_pool = ctx.enter_context(tc.tile_pool(name="m", bufs=2))
        sp_pool = ctx.enter_context(tc.tile_pool(name="sp", bufs=8))
        se_pool = ctx.enter_context(tc.tile_pool(name="se", bufs=4))
        o_pool = ctx.enter_context(tc.tile_pool(name="o", bufs=4))
        st_pool = ctx.enter_context(tc.tile_pool(name="st", bufs=st_bufs, space="PSUM"))
        ot_pool = ctx.enter_context(tc.tile_pool(name="otp", bufs=2, space="PSUM"))

        unit = 0
        loop_cm = tc.For_i(0, loop_n, 1) if loop_n > 1 else contextlib.nullcontext()
        with loop_cm:
          for p in range(n_pairs):
            qt_t = qt_pool.tile([128, sq], BF16)
            nc.sync.dma_start(out=qt_t[:d], in_=qt[p])
            kt_t = kt_pool.tile([128, sk], BF16)
            nc.sync.dma_start(out=kt_t[:d], in_=kt[p])
            v_t = v_pool.tile([128, nkc * d], BF16)
            nc.sync.dma_start(out=v_t[:], in_=v[p])

            for qc in range(nqc):
                m_t = m_pool.tile([128, nkc * qn], U8)
                nc.sync.dma_start(out=m_t[:], in_=mt[p, qc])
                ot_ps = ot_pool.tile([128, qn], F32)

                for kc in range(nkc):
                    st = st_pool.tile([128, qn], F32)
                    nc.tensor.matmul(
                        st[:],
                        kt_t[:d, kc * 128 : (kc + 1) * 128],
                        qt_t[:d, qc * qn : (qc + 1) * qn],
                        start=True,
                        stop=True,
                    )
                    m_sl = m_t[:, kc * qn : (kc + 1) * qn].bitcast(FP8)
                    sp = sp_pool.tile([128, qn], BF16)
                    if unit % a_every == a_every - 1:
                        se = se_pool.tile([128, qn], BF16)
                        nc.scalar.copy(se[:], st[:])
                        nc.gpsimd.tensor_mul(sp[:], se[:], m_sl)
                    else:
                        nc.vector.tensor_mul(sp[:], st[:], m_sl)
                    unit += 1
                    nc.tensor.matmul(
                        ot_ps[:d],
                        v_t[:, kc * d : (kc + 1) * d],
                        sp[:],
                        start=(kc == 0),
                        stop=(kc == nkc - 1),
                    )

                o_t = o_pool.tile([128, qn], BF16)
                nc.scalar.copy(o_t[:d], ot_ps[:d])
                nc.sync.dma_start(out=ot[p, qc], in_=o_t[:d])


def build_nc(n_pairs=PAIRS_PER_CORE, sq=SQ, sk=SK, d=D, qn=512, variant="v1", repeat=1,
             loop_n=1, a_every=3):
    if variant == "v5":
        nc = bacc.Bacc("TRN2", target_bir_lowering=False, debug=False)
        qt = nc.declare_dram_parameter("qt", [n_pairs, d, sq], BF16, isOutput=False)
        kt = nc.declare_dram_parameter("kt", [n_pairs, d, sk], BF16, isOutput=False)
        v = nc.declare_dram_parameter("v", [n_pairs, 128, (sk // 128) * d], BF16, isOutput=False)
        mt = nc.declare_dram_parameter("mt", [n_pairs, sq // qn, 128, (sk // 128) * qn], U8, isOutput=False)
        ot = nc.declare_dram_parameter("ot", [n_pairs, sq // qn, d, qn], BF16, isOutput=True)
        with tile.TileContext(nc) as tc:
            emit_body_v5(nc, tc, ot, qt, kt, v, mt, n_pairs, sq, sk, d, qn,
                         loop_n=loop_n, a_every=a_every)
        nc.compile()
        return nc
    if variant == "v4":
        nc = bacc.Bacc("TRN2", target_bir_lowering=False, debug=False)
        qt = nc.declare_dram_parameter("qt", [n_pairs, d, sq], BF16, isOutput=False)
        kt = nc.declare_dram_parameter("kt", [n_pairs, d, sk], BF16, isOutput=False)
        v = nc.declare_dram_parameter("v", [n_pairs, 128, (sk // 128) * d], BF16, isOutput=False)
        mt = nc.declare_dram_parameter("mt", [n_pairs, sq // qn, 128, (sk // 128) * qn], U8, isOutput=False)
        ot = nc.declare_dram_parameter("ot", [n_pairs, sq // qn, d, qn], BF16, isOutput=True)
        with tile.TileContext(nc) as tc:
            emit_body_v4(nc, tc, ot, qt, kt, v, mt, n_pairs, sq, sk, d, qn,
                         loop_n=loop_n)
        nc.compile()
        return nc
    nc = bacc.Bacc("TRN2", target_bir_lowering=False, debug=False)
    mmdt = F32R if variant == "v1" else BF16
    vdt = mmdt
    qt = nc.declare_dram_parameter("qt", [n_pairs, d, sq], mmdt, isOutput=False)
    kt = nc.declare_dram_parameter("kt", [n_pairs, d, sk], mmdt, isOutput=False)
    v = nc.declare_dram_parameter("v", [n_pairs, 128, (sk // 128) * d], vdt, isOutput=False)
    mt = nc.declare_dram_parameter("mt", [n_pairs, sk, sq], U8, isOutput=False)
    ot = nc.declare_dram_parameter("ot", [n_pairs, d, sq], F32, isOutput=True)
    with tile.TileContext(nc) as tc:
        if variant in ("v1", "v3"):
            emit_body(nc, tc, ot, qt, kt, v, mt, n_pairs, sq, sk, d, qn, repeat=repeat,
                      loop_n=loop_n, mmdt=mmdt)
        else:
            emit_body_v2(nc, tc, ot, qt, kt, v, mt, n_pairs, sq, sk, d, qn,
                         repeat=repeat, loop_n=loop_n)
    nc.compile()
    return nc


def _prep_inputs(query, key, value, dropout_mask, variant="v1", qn=512):
    """Host-side marshaling into per-core input maps."""
    import ml_dtypes

    q = np.asarray(query, dtype=np.float32).reshape(PAIRS, SQ, D)
    k = np.asarray(key, dtype=np.float32).reshape(PAIRS, SK, D)
    vv = np.asarray(value, dtype=np.float32).reshape(PAIRS, SK, D)
    m = np.asarray(dropout_mask).reshape(PAIRS, SQ, SK)

    qt = np.ascontiguousarray(q.transpose(0, 2, 1))  # [PAIRS, D, SQ]
    kt = np.ascontiguousarray(k.transpose(0, 2, 1))  # [PAIRS, D, SK]
    # V * SCALE rearranged: vr[p][r][c*D+j] = V[c*128+r, j] * SCALE
    vr = (vv * np.float32(SCALE)).reshape(PAIRS, SK // 128, 128, D)
    vr = np.ascontiguousarray(vr.transpose(0, 2, 1, 3)).reshape(PAIRS, 128, (SK // 128) * D)
    if variant != "v1":
        qt = qt.astype(ml_dtypes.bfloat16)
        kt = kt.astype(ml_dtypes.bfloat16)
    if variant in ("v2", "v4", "v5"):
        vr = vr.astype(ml_dtypes.bfloat16)
    # mask^T as fp8 bytes
    mb = (m != 0).astype(np.uint8) * np.uint8(FP8_ONE)  # [PAIRS, SQ, SK] u8
    if variant in ("v4", "v5"):
        # [PAIRS, nqc, 128, nkc*qn]: mt[p,qc,part,kc*qn+j] = mask[q=qc*qn+j, k=kc*128+part]
        nqc, nkc = SQ // qn, SK // 128
        mbt = mb.reshape(PAIRS, nqc, qn, nkc, 128)
        mbt = np.ascontiguousarray(mbt.transpose(0, 1, 4, 3, 2))  # [P,nqc,128,nkc,qn]
        mbt = mbt.reshape(PAIRS, nqc, 128, nkc * qn)
    else:
        mbt = np.ascontiguousarray(mb.transpose(0, 2, 1))  # [PAIRS, SK, SQ]

    in_maps = []
    for c in range(N_CORES):
        s = slice(c * PAIRS_PER_CORE, (c + 1) * PAIRS_PER_CORE)
        in_maps.append(
            {
                "qt": qt[s],
                "kt": kt[s],
                "v": vr[s],
                "mt": mbt[s],
            }
        )
    return in_maps


def kernel(query, key, value, dropout_mask):
    global LAST_RESULTS
    variant = os.environ.get("KERNEL_VARIANT", "v4")
    in_maps = _prep_inputs(query, key, value, dropout_mask, variant)
    nc = build_nc(variant=variant)
    res = run_bass_kernel_spmd(nc, in_maps, list(range(N_CORES)), trace=False)
    LAST_RESULTS = res
    outs = np.concatenate([r["ot"] for r in res.results], axis=0)
    if variant in ("v4", "v5"):
        # outs: [PAIRS, nqc, D, qn] (bf16) -> O^T [PAIRS, D, SQ]
        outs = np.asarray(outs, dtype=np.float32)
        outs = outs.transpose(0, 2, 1, 3).reshape(PAIRS, D, SQ)
    out = outs.transpose(0, 2, 1).reshape(B, H, SQ, D)
    return np.ascontiguousarray(out.astype(np.float32, copy=False))



# revision 15
# speedup vs baseline: 12.9404x; 2.1481x over previous
"""Trainium2 Bass kernel for masked-dropout attention-score matmul.

Computes, for q/k/v [B,H,S,D] and an int32 0/1 keep-mask [B,H,S,S]:

    out = ((q @ k^T) * sqrt(D) * 2 * mask) @ v        (2 = 1/(1-p_drop))

Strategy (8 NeuronCores, SPMD, no collectives) — shipped variant "v4":
  - Shard the 32 (b,h) pairs 4-per-core.
  - Per pair, qc-outer / kc-inner: compute S^T = K @ Q^T on the PE in
    [128,512] tiles (moving operand is Q^T so no on-chip transpose),
    apply the mask fused into the mandatory PSUM->SBUF eviction on the
    DVE (one tensor_mul per tile: PSUM f32 x fp8 mask -> bf16), and
    accumulate O^T = V^T @ S'^T on the PE into a single-bank [128,512]
    PSUM chunk (bufs=2, so evictions fully overlap the next chunk).
  - All matmul I/O is bf16 (q/k/v shipped bf16, scores bf16, out bf16 with
    host upcast): bf16 matmuls are ~1.6x faster than f32r on this silicon
    and halve the q/k/v/out DMA. fp8/bit-packed masks measured worse:
    fp8 quantization (2.65% RMS) fails the 2e-2 gate on matmul operands,
    and bit-unpacking costs a second per-element DVE op (the eviction is
    the engine wall, so any second elementwise pass loses).
  - The scale (2*sqrt(D)) is folded into V on the host; the mask ships as
    fp8(0/1) bytes pre-arranged per (pair,qc) so every DMA is contiguous;
    mask chunks alternate between the SP and ACT hardware DGE queues
    (dualq) with a 3-deep prefetch to ride out HBM contention.
"""

import os
import sys

sys.path.insert(0, "/opt/trn_rl_repo")

import numpy as np

import concourse.bacc as bacc
import concourse.bass as bass
import concourse.mybir as mybir
import concourse.tile as tile
from concourse.bass_utils import run_bass_kernel_spmd

B, H, SQ, SK, D = 2, 16, 2048, 2048, 128
P_DROP = 0.5
SCALE = float(D) ** 0.5 / (1.0 - P_DROP)  # folded into V on the host
N_CORES = 8
PAIRS = B * H
PAIRS_PER_CORE = PAIRS // N_CORES

F32 = mybir.dt.float32
F32R = mybir.dt.float32r
FP8 = mybir.dt.float8e4
U8 = mybir.dt.uint8
BF16 = mybir.dt.bfloat16

FP8_ONE = 0x38  # float8_e4m3 encoding of 1.0

# module-level handle for test.py to inspect timing after a traced run
LAST_RESULTS = None


def emit_body(nc, tc, ot, qt, kt, v, mt, n_pairs, sq, sk, d=D, qn=512, repeat=1,
              loop_n=1, mmdt=F32R):
    """Emit the per-core program.

    APs (all on this core's DRAM):
      qt [n_pairs, d,  sq]  f32  : Q^T per pair
      kt [n_pairs, d,  sk]  f32  : K^T per pair
      v  [n_pairs, d?, ...]      : V rearranged to [128, (sk//128)*d], f32,
                                   v[p][r][c*d+j] = V[c*128+r, j] * SCALE
      mt [n_pairs, sk, sq]  u8   : mask^T as fp8 bytes (0x00 / 0x38)
      ot [n_pairs, d,  sq]  f32  : O^T output
    """
    nkc = sk // 128
    nqc = sq // qn

    import contextlib

    with contextlib.ExitStack() as ctx:
        qt_pool = ctx.enter_context(tc.tile_pool(name="qt", bufs=2))
        kt_pool = ctx.enter_context(tc.tile_pool(name="kt", bufs=2))
        v_pool = ctx.enter_context(tc.tile_pool(name="v", bufs=2))
        m_pool = ctx.enter_context(tc.tile_pool(name="m", bufs=4))
        sp_pool = ctx.enter_context(tc.tile_pool(name="sp", bufs=6))
        o_pool = ctx.enter_context(tc.tile_pool(name="o", bufs=2))
        st_pool = ctx.enter_context(tc.tile_pool(name="st", bufs=4, space="PSUM"))
        ot_pool = ctx.enter_context(tc.tile_pool(name="otp", bufs=1, space="PSUM"))

        loop_cm = tc.For_i(0, loop_n, 1) if loop_n > 1 else contextlib.nullcontext()
        with loop_cm:
          for p in [pp for _ in range(repeat) for pp in range(n_pairs)]:
            qt_t = qt_pool.tile([128, sq], mmdt)
            nc.sync.dma_start(out=qt_t[:d], in_=qt[p])
            kt_t = kt_pool.tile([128, sk], mmdt)
            nc.sync.dma_start(out=kt_t[:d], in_=kt[p])
            v_t = v_pool.tile([128, nkc * d], mmdt)
            nc.sync.dma_start(out=v_t[:], in_=v[p])

            ot_ps = ot_pool.tile([128, sq], F32)

            for kc in range(nkc):
                m_t = m_pool.tile([128, sq], U8)
                nc.sync.dma_start(out=m_t[:], in_=mt[p, kc * 128 : (kc + 1) * 128, :])

                for qc in range(nqc):
                    st = st_pool.tile([128, qn], F32)
                    nc.tensor.matmul(
                        st[:],
                        kt_t[:d, kc * 128 : (kc + 1) * 128],
                        qt_t[:d, qc * qn : (qc + 1) * qn],
                        start=True,
                        stop=True,
                    )
                    sp = sp_pool.tile([128, qn], mmdt)
                    nc.vector.tensor_mul(
                        sp[:],
                        st[:],
                        m_t[:, qc * qn : (qc + 1) * qn].bitcast(FP8),
                    )
                    nc.tensor.matmul(
                        ot_ps[:d, qc * qn : (qc + 1) * qn],
                        v_t[:, kc * d : (kc + 1) * d],
                        sp[:],
                        start=(kc == 0),
                        stop=(kc == nkc - 1),
                    )

            o_t = o_pool.tile([128, sq], F32)
            nc.scalar.copy(o_t[:d], ot_ps[:d])
            nc.sync.dma_start(out=ot[p], in_=o_t[:d])


def emit_body_v2(
    nc, tc, ot, qt, kt, v, mt, n_pairs, sq, sk, d=D, qn=512, gn=1024, fused_mod=(1, 4),
    repeat=1, loop_n=1,
):
    """Balanced-engine variant.

    Masking is split across three engines per [128, gn] score group:
      - fused path (idx % fused_mod[1] < fused_mod[0]): DVE multiplies
        PSUM f32 scores by the fp8 mask directly -> bf16 SBUF.
      - split path: ACT evicts PSUM f32 -> bf16 SBUF, GpSimd converts the
        fp8 mask -> bf16, DVE multiplies bf16 x bf16 in its 2x mode.
    Second matmul runs with bf16 moving operand at N=gn; V ships as bf16.
    """
    nkc = sk // 128
    ngc = sq // gn

    import contextlib

    with contextlib.ExitStack() as ctx:
        qt_pool = ctx.enter_context(tc.tile_pool(name="qt", bufs=2))
        kt_pool = ctx.enter_context(tc.tile_pool(name="kt", bufs=2))
        v_pool = ctx.enter_context(tc.tile_pool(name="v", bufs=2))
        m_pool = ctx.enter_context(tc.tile_pool(name="m", bufs=4))
        sp_pool = ctx.enter_context(tc.tile_pool(name="sp", bufs=6))
        se_pool = ctx.enter_context(tc.tile_pool(name="se", bufs=4))
        mb_pool = ctx.enter_context(tc.tile_pool(name="mb", bufs=4))
        o_pool = ctx.enter_context(tc.tile_pool(name="o", bufs=2))
        st_pool = ctx.enter_context(tc.tile_pool(name="st", bufs=2, space="PSUM"))
        ot_pool = ctx.enter_context(tc.tile_pool(name="otp", bufs=1, space="PSUM"))

        unit = 0
        loop_cm = tc.For_i(0, loop_n, 1) if loop_n > 1 else contextlib.nullcontext()
        with loop_cm:
          for p in [pp for _ in range(repeat) for pp in range(n_pairs)]:
            qt_t = qt_pool.tile([128, sq], F32R)
            nc.sync.dma_start(out=qt_t[:d], in_=qt[p])
            kt_t = kt_pool.tile([128, sk], F32R)
            nc.sync.dma_start(out=kt_t[:d], in_=kt[p])
            v_t = v_pool.tile([128, nkc * d], BF16)
            nc.sync.dma_start(out=v_t[:], in_=v[p])

            ot_ps = ot_pool.tile([128, sq], F32)

            for kc in range(nkc):
                m_t = m_pool.tile([128, sq], U8)
                nc.sync.dma_start(out=m_t[:], in_=mt[p, kc * 128 : (kc + 1) * 128, :])

                for g in range(ngc):
                    st = st_pool.tile([128, gn], F32)
                    for j in range(gn // qn):
                        c0 = g * gn + j * qn
                        nc.tensor.matmul(
                            st[:, j * qn : (j + 1) * qn],
                            kt_t[:d, kc * 128 : (kc + 1) * 128],
                            qt_t[:d, c0 : c0 + qn],
                            start=True,
                            stop=True,
                        )
                    m_sl = m_t[:, g * gn : (g + 1) * gn].bitcast(FP8)
                    sp = sp_pool.tile([128, gn], BF16)
                    if unit % fused_mod[1] < fused_mod[0]:
                        nc.vector.tensor_mul(sp[:], st[:], m_sl)
                    else:
                        se = se_pool.tile([128, gn], BF16)
                        nc.scalar.copy(se[:], st[:])
                        mb = mb_pool.tile([128, gn], BF16)
                        nc.gpsimd.tensor_copy(mb[:], m_sl)
                        nc.vector.tensor_mul(sp[:], se[:], mb[:])
                    unit += 1
                    for j in range(gn // qn):
                        c0 = g * gn + j * qn
                        nc.tensor.matmul(
                            ot_ps[:d, c0 : c0 + qn],
                            v_t[:, kc * d : (kc + 1) * d],
                            sp[:, j * qn : (j + 1) * qn],
                            start=(kc == 0),
                            stop=(kc == nkc - 1),
                        )

            o_t = o_pool.tile([128, sq], F32)
            nc.scalar.copy(o_t[:d], ot_ps[:d])
            nc.sync.dma_start(out=ot[p], in_=o_t[:d])


def emit_body_v4(nc, tc, ot, qt, kt, v, mt, n_pairs, sq, sk, d=D, qn=512,
                 loop_n=1, st_bufs=6, m_bufs=2, diag=None, dualq=False,
                 ot_bufs=2, sp_bufs=6):
    """qc-outer / kc-inner: PSUM out chunk is one [128, qn] bank (bufs=2),
    evictions overlap compute; all matmul I/O bf16; mask fp8 fused on DVE.

    APs:
      qt [n_pairs, d, sq]   bf16 : Q^T
      kt [n_pairs, d, sk]   bf16 : K^T
      v  [n_pairs, 128, (sk//128)*d] bf16 : V * SCALE, kc-chunked
      mt [n_pairs, sq//qn, 128, (sk//128)*qn] u8 : mask fp8 bytes, laid out
         mt[p, qc, part, kc*qn + j] = mask[k = kc*128+part, q = qc*qn+j]
      ot [n_pairs, sq//qn, d, qn] bf16 : O^T chunks
    """
    nkc = sk // 128
    nqc = sq // qn

    import contextlib

    with contextlib.ExitStack() as ctx:
        qt_pool = ctx.enter_context(tc.tile_pool(name="qt", bufs=2))
        kt_pool = ctx.enter_context(tc.tile_pool(name="kt", bufs=2))
        v_pool = ctx.enter_context(tc.tile_pool(name="v", bufs=2))
        m_pool = ctx.enter_context(tc.tile_pool(name="m", bufs=m_bufs))
        sp_pool = ctx.enter_context(tc.tile_pool(name="sp", bufs=sp_bufs))
        o_pool = ctx.enter_context(tc.tile_pool(name="o", bufs=4))
        st_pool = ctx.enter_context(tc.tile_pool(name="st", bufs=st_bufs, space="PSUM"))
        ot_pool = ctx.enter_context(tc.tile_pool(name="otp", bufs=ot_bufs, space="PSUM"))

        sp_const = None
        if diag == "nomul":
            sp_const = qt_pool.tile([128, qn], BF16)
            nc.vector.memset(sp_const[:], 0.5)

        loop_cm = tc.For_i(0, loop_n, 1) if loop_n > 1 else contextlib.nullcontext()
        with loop_cm:
          for p in range(n_pairs):
            qt_t = qt_pool.tile([128, sq], BF16)
            nc.sync.dma_start(out=qt_t[:d], in_=qt[p])
            kt_t = kt_pool.tile([128, sk], BF16)
            nc.sync.dma_start(out=kt_t[:d], in_=kt[p])
            v_t = v_pool.tile([128, nkc * d], BF16)
            nc.sync.dma_start(out=v_t[:], in_=v[p])

            for qc in range(nqc):
                m_t = m_pool.tile([128, nkc * qn], U8)
                m_eng = nc.scalar if (dualq and qc % 2 == 1) else nc.sync
                m_eng.dma_start(out=m_t[:], in_=mt[p, qc])
                ot_ps = ot_pool.tile([128, qn], F32)

                nmc = (qn + 511) // 512  # matmul moving-dim chunks
                mn = qn // nmc
                for kc in range(nkc):
                    st = st_pool.tile([128, qn], F32)
                    for j in range(nmc):
                        nc.tensor.matmul(
                            st[:, j * mn : (j + 1) * mn],
                            kt_t[:d, kc * 128 : (kc + 1) * 128],
                            qt_t[:d, qc * qn + j * mn : qc * qn + (j + 1) * mn],
                            start=True,
                            stop=True,
                        )
                    if diag == "nomul":
                        sp = sp_const
                    else:
                        sp = sp_pool.tile([128, qn], BF16)
                        nc.vector.tensor_mul(
                            sp[:],
                            st[:],
                            m_t[:, kc * qn : (kc + 1) * qn].bitcast(FP8),
                        )
                    if diag == "nomm2":
                        if kc == 0:
                            nc.tensor.matmul(ot_ps[:d, :mn], v_t[:, :d], sp[:, :mn],
                                             start=True, stop=True)
                    else:
                        for j in range(nmc):
                            nc.tensor.matmul(
                                ot_ps[:d, j * mn : (j + 1) * mn],
                                v_t[:, kc * d : (kc + 1) * d],
                                sp[:, j * mn : (j + 1) * mn],
                                start=(kc == 0),
                                stop=(kc == nkc - 1),
                            )

                o_t = o_pool.tile([128, qn], BF16)
                nc.scalar.copy(o_t[:d], ot_ps[:d])
                nc.sync.dma_start(out=ot[p, qc], in_=o_t[:d])


def emit_body_v5(nc, tc, ot, qt, kt, v, mt, n_pairs, sq, sk, d=D, qn=512,
                 loop_n=1, a_every=3, st_bufs=5):
    """3-engine masked eviction, qc-outer/kc-inner.

    Unit classes (u = global unit counter):
      D (default): DVE fused mul-evict  sp_f32r = st_PSUM * mask_fp8
      A (u % a_every == a_every-1): ACT copy st_PSUM -> se_f32, then
        GpSimd mul sp_f32r = se * mask_fp8  (keeps DVE free)
    MM1 bf16 (qt,kt); MM2 f32r moving (sp) x f32r stationary (v).
    """
    nkc = sk // 128
    nqc = sq // qn

    import contextlib

    with contextlib.ExitStack() as ctx:
        qt_pool = ctx.enter_context(tc.tile_pool(name="qt", bufs=2))
        kt_pool = ctx.enter_context(tc.tile_pool(name="kt", bufs=2))
        v_pool = ctx.enter_context(tc.tile_pool(name="v", bufs=2))
        m_pool = ctx.enter_context(tc.tile_pool(name="m", bufs=2))
        sp_pool = ctx.enter_context(tc.tile_pool(name="sp", bufs=8))
        se_pool = ctx.enter_context(tc.tile_pool(name="se", bufs=4))
        o_pool = ctx.enter_context(tc.tile_pool(name="o", bufs=4))
        st_pool = ctx.enter_context(tc.tile_pool(name="st", bufs=st_bufs, space="PSUM"))
        ot_pool = ctx.enter_context(tc.tile_pool(name="otp", bufs=2, space="PSUM"))

        unit = 0
        loop_cm = tc.For_i(0, loop_n, 1) if loop_n > 1 else contextlib.nullcontext()
        with loop_cm:
          for p in range(n_pairs):
            qt_t = qt_pool.tile([128, sq], BF16)
            nc.sync.dma_start(out=qt_t[:d], in_=qt[p])
            kt_t = kt_pool.tile([128, sk], BF16)
            nc.sync.dma_start(out=kt_t[:d], in_=kt[p])
            v_t = v_pool.tile([128, nkc * d], BF16)
            nc.sync.dma_start(out=v_t[:], in_=v[p])

            for qc in range(nqc):
                m_t = m_pool.tile([128, nkc * qn], U8)
                nc.sync.dma_start(out=m_t[:], in_=mt[p, qc])
                ot_ps = ot_pool.tile([128, qn], F32)

                for kc in range(nkc):
                    st = st_pool.tile([128, qn], F32)
                    nc.tensor.matmul(
                        st[:],
                        kt_t[:d, kc * 128 : (kc + 1) * 128],
                        qt_t[:d, qc * qn : (qc + 1) * qn],
                        start=True,
                        stop=True,
                    )
                    m_sl = m_t[:, kc * qn : (kc + 1) * qn].bitcast(FP8)
                    sp = sp_pool.tile([128, qn], BF16)
                    if unit % a_every == a_every - 1:
                        se = se_pool.tile([128, qn], BF16)
                        nc.scalar.copy(se[:], st[:])
                        nc.gpsimd.tensor_mul(sp[:], se[:], m_sl)
                    else:
                        nc.vector.tensor_mul(sp[:], st[:], m_sl)
                    unit += 1
                    nc.tensor.matmul(
                        ot_ps[:d],
                        v_t[:, kc * d : (kc + 1) * d],
                        sp[:],
                        start=(kc == 0),
                        stop=(kc == nkc - 1),
                    )

                o_t = o_pool.tile([128, qn], BF16)
                nc.scalar.copy(o_t[:d], ot_ps[:d])
                nc.sync.dma_start(out=ot[p, qc], in_=o_t[:d])


def build_nc(n_pairs=PAIRS_PER_CORE, sq=SQ, sk=SK, d=D, qn=512, variant="v1", repeat=1,
             loop_n=1, a_every=3, diag=None, st_bufs=None, m_bufs=2, ot_bufs=2,
             dualq=False, sp_bufs=6):
    if st_bufs is None:
        st_bufs = 6 if qn <= 512 else 2
    if variant == "v5":
        nc = bacc.Bacc("TRN2", target_bir_lowering=False, debug=False)
        qt = nc.declare_dram_parameter("qt", [n_pairs, d, sq], BF16, isOutput=False)
        kt = nc.declare_dram_parameter("kt", [n_pairs, d, sk], BF16, isOutput=False)
        v = nc.declare_dram_parameter("v", [n_pairs, 128, (sk // 128) * d], BF16, isOutput=False)
        mt = nc.declare_dram_parameter("mt", [n_pairs, sq // qn, 128, (sk // 128) * qn], U8, isOutput=False)
        ot = nc.declare_dram_parameter("ot", [n_pairs, sq // qn, d, qn], BF16, isOutput=True)
        with tile.TileContext(nc) as tc:
            emit_body_v5(nc, tc, ot, qt, kt, v, mt, n_pairs, sq, sk, d, qn,
                         loop_n=loop_n, a_every=a_every)
        nc.compile()
        return nc
    if variant == "v4":
        nc = bacc.Bacc("TRN2", target_bir_lowering=False, debug=False)
        qt = nc.declare_dram_parameter("qt", [n_pairs, d, sq], BF16, isOutput=False)
        kt = nc.declare_dram_parameter("kt", [n_pairs, d, sk], BF16, isOutput=False)
        v = nc.declare_dram_parameter("v", [n_pairs, 128, (sk // 128) * d], BF16, isOutput=False)
        mt = nc.declare_dram_parameter("mt", [n_pairs, sq // qn, 128, (sk // 128) * qn], U8, isOutput=False)
        ot = nc.declare_dram_parameter("ot", [n_pairs, sq // qn, d, qn], BF16, isOutput=True)
        with tile.TileContext(nc) as tc:
            emit_body_v4(nc, tc, ot, qt, kt, v, mt, n_pairs, sq, sk, d, qn,
                         loop_n=loop_n, diag=diag, st_bufs=st_bufs, m_bufs=m_bufs,
                         dualq=dualq, ot_bufs=ot_bufs, sp_bufs=sp_bufs)
        nc.compile()
        return nc
    nc = bacc.Bacc("TRN2", target_bir_lowering=False, debug=False)
    mmdt = F32R if variant == "v1" else BF16
    vdt = mmdt
    qt = nc.declare_dram_parameter("qt", [n_pairs, d, sq], mmdt, isOutput=False)
    kt = nc.declare_dram_parameter("kt", [n_pairs, d, sk], mmdt, isOutput=False)
    v = nc.declare_dram_parameter("v", [n_pairs, 128, (sk // 128) * d], vdt, isOutput=False)
    mt = nc.declare_dram_parameter("mt", [n_pairs, sk, sq], U8, isOutput=False)
    ot = nc.declare_dram_parameter("ot", [n_pairs, d, sq], F32, isOutput=True)
    with tile.TileContext(nc) as tc:
        if variant in ("v1", "v3"):
            emit_body(nc, tc, ot, qt, kt, v, mt, n_pairs, sq, sk, d, qn, repeat=repeat,
                      loop_n=loop_n, mmdt=mmdt)
        else:
            emit_body_v2(nc, tc, ot, qt, kt, v, mt, n_pairs, sq, sk, d, qn,
                         repeat=repeat, loop_n=loop_n)
    nc.compile()
    return nc


def _prep_inputs(query, key, value, dropout_mask, variant="v1", qn=512):
    """Host-side marshaling into per-core input maps."""
    import ml_dtypes

    q = np.asarray(query, dtype=np.float32).reshape(PAIRS, SQ, D)
    k = np.asarray(key, dtype=np.float32).reshape(PAIRS, SK, D)
    vv = np.asarray(value, dtype=np.float32).reshape(PAIRS, SK, D)
    m = np.asarray(dropout_mask).reshape(PAIRS, SQ, SK)

    qt = np.ascontiguousarray(q.transpose(0, 2, 1))  # [PAIRS, D, SQ]
    kt = np.ascontiguousarray(k.transpose(0, 2, 1))  # [PAIRS, D, SK]
    # V * SCALE rearranged: vr[p][r][c*D+j] = V[c*128+r, j] * SCALE
    vr = (vv * np.float32(SCALE)).reshape(PAIRS, SK // 128, 128, D)
    vr = np.ascontiguousarray(vr.transpose(0, 2, 1, 3)).reshape(PAIRS, 128, (SK // 128) * D)
    if variant != "v1":
        qt = qt.astype(ml_dtypes.bfloat16)
        kt = kt.astype(ml_dtypes.bfloat16)
    if variant in ("v2", "v4", "v5"):
        vr = vr.astype(ml_dtypes.bfloat16)
    # mask^T as fp8 bytes
    mb = (m != 0).astype(np.uint8) * np.uint8(FP8_ONE)  # [PAIRS, SQ, SK] u8
    if variant in ("v4", "v5"):
        # [PAIRS, nqc, 128, nkc*qn]: mt[p,qc,part,kc*qn+j] = mask[q=qc*qn+j, k=kc*128+part]
        nqc, nkc = SQ // qn, SK // 128
        mbt = mb.reshape(PAIRS, nqc, qn, nkc, 128)
        mbt = np.ascontiguousarray(mbt.transpose(0, 1, 4, 3, 2))  # [P,nqc,128,nkc,qn]
        mbt = mbt.reshape(PAIRS, nqc, 128, nkc * qn)
    else:
        mbt = np.ascontiguousarray(mb.transpose(0, 2, 1))  # [PAIRS, SK, SQ]

    in_maps = []
    for c in range(N_CORES):
        s = slice(c * PAIRS_PER_CORE, (c + 1) * PAIRS_PER_CORE)
        in_maps.append(
            {
                "qt": qt[s],
                "kt": kt[s],
                "v": vr[s],
                "mt": mbt[s],
            }
        )
    return in_maps


# Best-measured v4 build configuration (see exp_tune.py results).
V4_KW = dict(qn=512, m_bufs=3, dualq=True)


def kernel(query, key, value, dropout_mask):
    global LAST_RESULTS
    variant = os.environ.get("KERNEL_VARIANT", "v4")
    bkw = dict(V4_KW) if variant == "v4" else {}
    in_maps = _prep_inputs(query, key, value, dropout_mask, variant,
                           qn=bkw.get("qn", 512))
    nc = build_nc(variant=variant, **bkw)
    res = run_bass_kernel_spmd(nc, in_maps, list(range(N_CORES)), trace=False)
    LAST_RESULTS = res
    outs = np.concatenate([r["ot"] for r in res.results], axis=0)
    if variant in ("v4", "v5"):
        # outs: [PAIRS, nqc, D, qn] (bf16) -> O^T [PAIRS, D, SQ]
        outs = np.asarray(outs, dtype=np.float32)
        outs = outs.transpose(0, 2, 1, 3).reshape(PAIRS, D, SQ)
    out = outs.transpose(0, 2, 1).reshape(B, H, SQ, D)
    return np.ascontiguousarray(out.astype(np.float32, copy=False))

